# revision 5
# baseline (speedup 1.0000x reference)
"""Multi-head causal attention on 8 Trainium2 NeuronCores.

Sharding: data-parallel over batch (4) x tensor-parallel over heads (2 groups
of 8 heads). Each core computes a partial output [T, C] for one batch element
using its 8 heads; the host sums the two partials per batch element (the
"all-reduce after out_proj" done during unshard).

v2 layout (vs baseline): token-chunk t of the QKV projection is interleaved
with attention query-chunk j=t (all keys <= chunk t are ready), so the exp
work on the Scalar engine overlaps projection matmuls instead of serializing
behind them.  Causal raggedness is exploited: for query chunk j, diagonal key
block kb=4j+m streams only the N = 512-128m live query columns, the exp covers
only the live range, and the causal mask shrinks to one [128,128] triangle
multiply per diagonal block (on GpSimd, which is otherwise idle).  Softmax
normalization happens per query chunk (no end-of-kernel barrier), out_proj for
chunk j runs right after, and the output is DMA'd as bf16 (host does the final
f32 partial sum).

Per-core algorithm (layouts chosen so no on-device transposes are needed):
  inputs: xT [C, T] (x[b] transposed on host), Wq*0.125/Wk/Wv [C, 512],
          Wo [512, C], triangle mask [128, 128] bf16, sel [8, 512] f32r.
  QT = (Wq/8)^T @ x^T  [512, T]   (lhsT = Wq chunk, rhs = xT chunk)
  KT = Wk^T @ x^T      [512, T]
  V  = x @ Wv          [T, 512]   ones-augmented as vaug [T, 8 heads, 65]
  per head-pair p, query-chunk j, key-block kb (ragged causal):
     sT  = K_h[kb]^T @ Q_h[:, live]      [128, <=512] PSUM per half
     p   = exp(sT)  (skip-max softmax: |s| < ~9), one exp per kb tile
     p  *= triangle mask on the leading 128 cols of diagonal blocks
     av += V_aug[kb, h]^T @ p            [65, 512] PSUM; row 64 = denom
  per j: gather the 8 denom rows, reciprocal, sel-matmul broadcast,
  normalize aot in place, then out_proj of chunk j -> bf16 DMA out.
"""

import numpy as np
import ml_dtypes

_BF = ml_dtypes.bfloat16

import concourse.bass as bass
import concourse.bacc as bacc
import concourse.mybir as mybir
import concourse.tile as tile
from concourse import bass_utils

F32 = mybir.dt.float32
F32R = mybir.dt.float32r
BF16 = mybir.dt.bfloat16

B, T, C = 4, 2048, 1024
H, Dh = 16, 64
G = 2                 # head groups (tensor parallel)
HPG = H // G          # heads per group
GC = HPG * Dh         # group channels = 512
N_CORES = 8
TC = 512              # token chunk (projection and query chunks)
KB = 128              # key block
N_TC = T // TC        # 4
N_KB = T // KB        # 16
N_CC = C // 128       # contraction chunks over C = 8
N_GCB = GC // 128     # chan blocks in a group = 4


def build_program():
    nc = bacc.Bacc("TRN2", target_bir_lowering=False, debug=False)

    xT = nc.dram_tensor("xT", [C, T], BF16, kind="ExternalInput").ap()
    wq = nc.dram_tensor("wq", [C, GC], BF16, kind="ExternalInput").ap()
    wk = nc.dram_tensor("wk", [C, GC], BF16, kind="ExternalInput").ap()
    wv = nc.dram_tensor("wv", [C, GC], BF16, kind="ExternalInput").ap()
    wo = nc.dram_tensor("wo", [GC, C], BF16, kind="ExternalInput").ap()
    mask_in = nc.dram_tensor("mask", [KB, KB], BF16, kind="ExternalInput").ap()
    sel_in = nc.dram_tensor("sel", [8, 8 * Dh], F32R, kind="ExternalInput").ap()
    out = nc.dram_tensor("out", [T, C], BF16, kind="ExternalOutput").ap()

    with tile.TileContext(nc) as tc:
        with (
            tc.tile_pool(name="persist", bufs=1) as pp,
            tc.tile_pool(name="x_pool", bufs=2) as xp,
            tc.tile_pool(name="probs", bufs=4) as prp,
            tc.tile_pool(name="outs", bufs=2) as otp,
            tc.tile_pool(name="dc_pool", bufs=2) as dcp,
            tc.tile_pool(name="pj_psum", bufs=2, space="PSUM") as pjp,
            tc.tile_pool(name="sc_psum", bufs=2, space="PSUM") as scp,
            tc.tile_pool(name="av_psum", bufs=2, space="PSUM") as avp,
        ):
            qt = pp.tile([128, N_GCB, T], BF16)        # QT (chan%128, chan//128, tok)
            kt = pp.tile([128, N_GCB, T], BF16)
            vaug = pp.tile([128, N_KB, HPG, Dh + 1], BF16)
            aot = pp.tile([128, N_GCB, T], BF16)       # attn_outT
            msk = pp.tile([128, KB], BF16)
            sel = pp.tile([8, 8 * Dh], F32R)
            # softmax denominator staging: slot (j, idx8) at partition
            # 32*(idx8//3), column 3*j + idx8%3 (engine APs start at 0/32/64)
            dens = pp.tile([128, 12, TC], F32)

            wqs = pp.tile([128, N_CC, GC], BF16)
            wks = pp.tile([128, N_CC, GC], BF16)
            wvs = pp.tile([128, N_CC, GC], BF16)
            wos = pp.tile([128, N_GCB, C], BF16)

            # DMA order = priority order for the startup window.
            for oc in range(N_GCB):      # wq in per-oc slices: first matmul
                nc.sync.dma_start(       # can start after 0.25 MB
                    wqs[:, :, oc * 128:(oc + 1) * 128],
                    wq.rearrange("(kc p) n -> p kc n", p=128)[
                        :, :, oc * 128:(oc + 1) * 128
                    ],
                )
            xts = []
            for t in range(N_TC):
                xts.append(
                    xp.tile([128, N_CC, TC], BF16, tag="xt", name=f"xt{t}")
                )
            nc.sync.dma_start(
                xts[0][:],
                xT[:, 0:TC].rearrange("(kc p) n -> p kc n", p=128),
            )
            nc.sync.dma_start(wks[:], wk.rearrange("(kc p) n -> p kc n", p=128))
            nc.sync.dma_start(wvs[:], wv.rearrange("(kc p) n -> p kc n", p=128))
            nc.sync.dma_start(msk[:], mask_in)
            nc.sync.dma_start(sel[:], sel_in)
            nc.sync.dma_start(wos[:], wo.rearrange("(cb p) n -> p cb n", p=128))
            nc.vector.memset(vaug[:, :, :, Dh:], 1.0)

            for t in range(N_TC):
                # -------- phase 2 unit: project token chunk t ------------
                xt = xts[t]
                if t + 1 < N_TC:
                    nc.sync.dma_start(
                        xts[t + 1][:],
                        xT[:, (t + 1) * TC:(t + 2) * TC].rearrange(
                            "(kc p) n -> p kc n", p=128
                        ),
                    )
                for oc in range(N_GCB):      # QT and KT column blocks
                    for w_s, dst, eng in (
                        (wqs, qt, nc.scalar), (wks, kt, nc.vector)
                    ):
                        ps = pjp.tile([128, TC], F32, tag="pj")
                        for kc in range(N_CC):
                            nc.tensor.matmul(
                                ps[:],
                                w_s[:, kc, oc * 128:(oc + 1) * 128],
                                xt[:, kc, :],
                                start=(kc == 0),
                                stop=(kc == N_CC - 1),
                            )
                        dslc = dst[:, oc, t * TC:(t + 1) * TC]
                        if eng is nc.scalar:
                            nc.scalar.copy(dslc, ps[:])
                        else:
                            nc.vector.tensor_copy(dslc, ps[:])
                for tb in range(TC // 128):  # V token blocks
                    ps = pjp.tile([128, GC], F32, tag="pj")
                    for kc in range(N_CC):
                        nc.tensor.matmul(
                            ps[:],
                            xt[:, kc, tb * 128:(tb + 1) * 128],
                            wvs[:, kc, :],
                            start=(kc == 0),
                            stop=(kc == N_CC - 1),
                        )
                    nc.vector.tensor_copy(
                        vaug[:, t * 4 + tb, :, :Dh],
                        ps.rearrange("p (h d) -> p h d", h=HPG),
                    )

                # -------- phase 3 unit: attention for query chunk j=t ----
                j = t
                qslc = slice(j * TC, (j + 1) * TC)
                for p in range(HPG // 2):    # head pairs: rows 0:64 / 64:128
                    avs = [
                        avp.tile([Dh + 1, TC], F32, tag="av", name=f"av{i}")
                        for i in range(2)
                    ]
                    nkb = 4 * j + 4
                    for kb in range(nkb):
                        m = kb - 4 * j       # >=0 on diagonal blocks
                        c0 = m * 128 if m > 0 else 0
                        w = TC - c0          # live query columns per half
                        # both heads' score tiles packed [c0:512 | 512:512+w]
                        # in one 2-bank PSUM tile -> single exp op per kb
                        sc = scp.tile([128, 2 * TC], F32, tag="sc")
                        for half in range(2):
                            p0 = half * Dh
                            dst = (
                                sc[:, c0:TC] if half == 0
                                else sc[:, TC:TC + w]
                            )
                            nc.tensor.matmul(
                                dst,
                                kt[p0:p0 + Dh, p, kb * KB:(kb + 1) * KB],
                                qt[p0:p0 + Dh, p, j * TC + c0:(j + 1) * TC],
                                start=True,
                                stop=True,
                            )
                        pr = prp.tile([128, 2 * TC], BF16, tag="pr")
                        nc.scalar.activation(
                            pr[:, c0:TC + w], sc[:, c0:TC + w],
                            mybir.ActivationFunctionType.Exp,
                        )
                        if m >= 0:
                            # causal triangle on the leading 128 live cols
                            for half in range(2):
                                base = c0 if half == 0 else TC
                                nc.gpsimd.tensor_mul(
                                    pr[:, base:base + KB],
                                    pr[:, base:base + KB],
                                    msk[:],
                                )
                        for half in range(2):
                            src = (
                                pr[:, c0:TC] if half == 0
                                else pr[:, TC:TC + w]
                            )
                            nc.tensor.matmul(
                                avs[half][:, c0:],
                                vaug[:, kb, 2 * p + half, :],
                                src,
                                start=(kb == 0),
                                stop=(kb == nkb - 1),
                            )
                    for half in range(2):
                        p0 = half * Dh
                        idx8 = 2 * p + half
                        nc.vector.tensor_copy(
                            aot[p0:p0 + Dh, p, qslc], avs[half][:Dh, :]
                        )
                        db, dc = 32 * (idx8 // 3), 3 * j + idx8 % 3
                        nc.vector.tensor_copy(
                            dens[db:db + 1, dc, :], avs[half][Dh:Dh + 1, :]
                        )

                # per-chunk softmax normalization + out_proj
                dcomp = dcp.tile([8, TC], F32, tag="dc")
                for b3 in range(3):
                    lo, n = 3 * b3, min(3, 8 - 3 * b3)
                    nc.sync.dma_start(
                        dcomp[lo:lo + n, :],
                        dens[32 * b3:32 * b3 + 1, 3 * j:3 * j + n, :],
                    )
                rec = dcp.tile([8, TC], F32R, tag="rec")
                with nc.allow_low_precision(
                    reason="fp32r reciprocal feeds bcast matmul"
                ):
                    nc.vector.reciprocal(rec[:], dcomp[:])
                for p in range(HPG // 2):
                    for half in range(2):
                        p0 = half * Dh
                        idx8 = 2 * p + half
                        bc = pjp.tile([Dh, TC], F32, tag="pj")
                        nc.tensor.matmul(
                            bc[:],
                            sel[:, idx8 * Dh:(idx8 + 1) * Dh],
                            rec[:],
                            start=True, stop=True,
                        )
                        nc.vector.tensor_mul(
                            aot[p0:p0 + Dh, p, qslc],
                            aot[p0:p0 + Dh, p, qslc],
                            bc[:],
                        )
                for tb in range(4 * j, 4 * j + 4):
                    ot = otp.tile([128, C], BF16, tag="ot")
                    for oc in range(C // TC):
                        ps = pjp.tile([128, TC], F32, tag="pj")
                        for cc in range(N_GCB):
                            nc.tensor.matmul(
                                ps[:],
                                aot[:, cc, tb * 128:(tb + 1) * 128],
                                wos[:, cc, oc * TC:(oc + 1) * TC],
                                start=(cc == 0),
                                stop=(cc == N_GCB - 1),
                            )
                        nc.vector.tensor_copy(
                            ot[:, oc * TC:(oc + 1) * TC], ps[:]
                        )
                    nc.sync.dma_start(out[tb * 128:(tb + 1) * 128, :], ot[:])

    nc.compile()
    return nc


_CACHE = {}


def _make_mask():
    m = np.zeros((KB, KB), np.float32)
    for dk in range(KB):
        m[dk, dk:] = 1.0
    return m.astype(_BF)


def _make_sel():
    s = np.zeros((8, 8 * Dh), np.float32)
    for i in range(8):
        s[i, i * Dh:(i + 1) * Dh] = 1.0
    return s


def make_in_maps(x, W_qkv, W_out):
    mask = _make_mask()
    sel = _make_sel()
    in_maps = []
    for core in range(N_CORES):
        b, g = divmod(core, G)
        cs = slice(g * GC, (g + 1) * GC)
        in_maps.append({
            "xT": np.ascontiguousarray(x[b].T).astype(_BF),
            "wq": np.ascontiguousarray(W_qkv[:, cs] * 0.125).astype(_BF),
            "wk": np.ascontiguousarray(
                W_qkv[:, C + g * GC:C + (g + 1) * GC]).astype(_BF),
            "wv": np.ascontiguousarray(
                W_qkv[:, 2 * C + g * GC:2 * C + (g + 1) * GC]).astype(_BF),
            "wo": np.ascontiguousarray(W_out[cs, :]).astype(_BF),
            "mask": mask,
            "sel": sel,
        })
    return in_maps


def kernel(x, W_qkv, W_out):
    x = np.ascontiguousarray(np.asarray(x, dtype=np.float32))
    W_qkv = np.asarray(W_qkv, dtype=np.float32)
    W_out = np.asarray(W_out, dtype=np.float32)

    if "nc" not in _CACHE:
        _CACHE["nc"] = build_program()
    nc = _CACHE["nc"]

    in_maps = make_in_maps(x, W_qkv, W_out)
    res = bass_utils.run_bass_kernel_spmd(nc, in_maps, core_ids=list(range(N_CORES)))

    out = np.empty((B, T, C), np.float32)
    for b in range(B):
        out[b] = res.results[G * b]["out"].astype(np.float32)
        for g in range(1, G):
            out[b] += res.results[G * b + g]["out"].astype(np.float32)
    return out


# revision 9
# speedup vs baseline: 1.0900x; 1.0900x over previous
"""Multi-head causal attention on 8 Trainium2 NeuronCores.

Sharding: data-parallel over batch (4) x tensor-parallel over heads (2 groups
of 8 heads). Each core computes a partial output [T, C] for one batch element
using its 8 heads; the host sums the two partials per batch element (the
"all-reduce after out_proj" done during unshard).

v3 structure: the kernel is a single loop over 4 token chunks.  Unit t
projects chunk t (Q/K/V), runs attention for query chunk j=t (whose keys are
exactly chunks 0..t), and then runs softmax-normalization + out_proj for
chunk j=t-1 (software pipelining: the denominator-gather DMA + reciprocal of
chunk j run during unit j+1's matmuls, so the PE never waits on them).
Causal raggedness: for query chunk j, diagonal key block kb=4j+m streams
only the N = 512-128m live query columns, exp covers only the live range,
and the causal mask is one [128,128] triangle multiply per diagonal block.
Output is DMA'd bf16 (host sums the two group partials in f32).

Per-core layouts (no on-device transposes needed):
  inputs: xT [C, T] (x[b] transposed on host), Wq*0.125/Wk/Wv [C, 512],
          Wo [512, C], triangle mask [128, 128] bf16, sel [8, 512] f32r.
  QT = (Wq/8)^T @ x^T  [512, T]   (lhsT = Wq chunk, rhs = xT chunk)
  KT = Wk^T @ x^T      [512, T]
  V  = x @ Wv          [T, 512]   ones-augmented as vaug [T, 8 heads, 65]
  per head-pair p, query-chunk j, key-block kb (ragged causal):
     sT  = K_h[kb]^T @ Q_h[:, live]      [128, <=512] PSUM per half
     p   = exp(sT)  (skip-max softmax: |s| < ~9), one exp per kb tile
     p  *= triangle mask on the leading 128 cols of diagonal blocks
     av += V_aug[kb, h]^T @ p            [65, 512] PSUM; row 64 = denom
  per j: gather the 8 denom rows, reciprocal_approx_fast, sel-matmul
  broadcast, normalize aot in place, out_proj of chunk j -> bf16 DMA out.
"""

import numpy as np
import ml_dtypes

_BF = ml_dtypes.bfloat16

import concourse.bass as bass
import concourse.bacc as bacc
import concourse.mybir as mybir
import concourse.tile as tile
from concourse import bass_utils

F32 = mybir.dt.float32
F32R = mybir.dt.float32r
BF16 = mybir.dt.bfloat16

B, T, C = 4, 2048, 1024
H, Dh = 16, 64
G = 2                 # head groups (tensor parallel)
HPG = H // G          # heads per group
GC = HPG * Dh         # group channels = 512
N_CORES = 8
TC = 512              # token chunk (projection and query chunks)
KB = 128              # key block
N_TC = T // TC        # 4
N_KB = T // KB        # 16
N_CC = C // 128       # contraction chunks over C = 8
N_GCB = GC // 128     # chan blocks in a group = 4


def build_program():
    nc = bacc.Bacc("TRN2", target_bir_lowering=False, debug=False)

    xT = nc.dram_tensor("xT", [C, T], BF16, kind="ExternalInput").ap()
    wq = nc.dram_tensor("wq", [C, GC], BF16, kind="ExternalInput").ap()
    wk = nc.dram_tensor("wk", [C, GC], BF16, kind="ExternalInput").ap()
    wv = nc.dram_tensor("wv", [C, GC], BF16, kind="ExternalInput").ap()
    wo = nc.dram_tensor("wo", [GC, C], BF16, kind="ExternalInput").ap()
    mask_in = nc.dram_tensor("mask", [KB, KB], BF16, kind="ExternalInput").ap()
    sel_in = nc.dram_tensor("sel", [8, 8 * Dh], F32R, kind="ExternalInput").ap()
    out = nc.dram_tensor("out", [T, C], BF16, kind="ExternalOutput").ap()

    with tile.TileContext(nc) as tc:
        with (
            tc.tile_pool(name="persist", bufs=1) as pp,
            tc.tile_pool(name="x_pool", bufs=2) as xp,
            tc.tile_pool(name="probs", bufs=4) as prp,
            tc.tile_pool(name="outs", bufs=2) as otp,
            tc.tile_pool(name="dc_pool", bufs=2) as dcp,
            tc.tile_pool(name="pj_psum", bufs=2, space="PSUM") as pjp,
            tc.tile_pool(name="sc_psum", bufs=2, space="PSUM") as scp,
            tc.tile_pool(name="av_psum", bufs=2, space="PSUM") as avp,
        ):
            qt = pp.tile([128, N_GCB, T], BF16)        # QT (chan%128, chan//128, tok)
            kt = pp.tile([128, N_GCB, T], BF16)
            vaug = pp.tile([128, N_KB, HPG, Dh + 1], BF16)
            aot = pp.tile([128, N_GCB, T], BF16)       # attn_outT
            msk = pp.tile([128, KB], BF16)
            sel = pp.tile([8, 8 * Dh], F32R)
            # softmax denominator staging: slot (j, idx8) at partition
            # 32*(idx8//3), column 3*j + idx8%3 (engine APs start at 0/32/64)
            dens = pp.tile([128, 12, TC], F32)

            wqs = pp.tile([128, N_CC, GC], BF16)
            wks = pp.tile([128, N_CC, GC], BF16)
            wvs = pp.tile([128, N_CC, GC], BF16)
            wos = pp.tile([128, N_GCB, C], BF16)

            xts = []
            for t in range(N_TC):
                xts.append(
                    xp.tile([128, N_CC, TC], BF16, tag="xt", name=f"xt{t}")
                )

            # startup DMAs spread across engine queues so the first
            # transfers run concurrently
            nc.sync.dma_start(wqs[:], wq.rearrange("(kc p) n -> p kc n", p=128))
            nc.scalar.dma_start(
                xts[0][:], xT[:, 0:TC].rearrange("(kc p) n -> p kc n", p=128)
            )
            nc.gpsimd.dma_start(wks[:], wk.rearrange("(kc p) n -> p kc n", p=128))
            nc.sync.dma_start(wvs[:], wv.rearrange("(kc p) n -> p kc n", p=128))
            nc.sync.dma_start(msk[:], mask_in)
            nc.sync.dma_start(sel[:], sel_in)
            nc.sync.dma_start(wos[:], wo.rearrange("(cb p) n -> p cb n", p=128))
            nc.vector.memset(vaug[:, :, :, Dh:], 1.0)

            recs = {}

            def norm_outproj(j):
                qslc = slice(j * TC, (j + 1) * TC)
                rec = recs.pop(j)
                for p in range(HPG // 2):
                    for half in range(2):
                        p0 = half * Dh
                        idx8 = 2 * p + half
                        bc = pjp.tile([Dh, TC], F32, tag="pj", name="bc")
                        nc.tensor.matmul(
                            bc[:],
                            sel[:, idx8 * Dh:(idx8 + 1) * Dh],
                            rec[:],
                            start=True, stop=True,
                        )
                        nc.vector.tensor_mul(
                            aot[p0:p0 + Dh, p, qslc],
                            aot[p0:p0 + Dh, p, qslc],
                            bc[:],
                        )
                for tb in range(4 * j, 4 * j + 4):
                    ot = otp.tile([128, C], BF16, tag="ot", name="ot")
                    for oc in range(C // TC):
                        ps = pjp.tile([128, TC], F32, tag="pj", name="op")
                        for cc in range(N_GCB):
                            nc.tensor.matmul(
                                ps[:],
                                aot[:, cc, tb * 128:(tb + 1) * 128],
                                wos[:, cc, oc * TC:(oc + 1) * TC],
                                start=(cc == 0),
                                stop=(cc == N_GCB - 1),
                            )
                        nc.vector.tensor_copy(
                            ot[:, oc * TC:(oc + 1) * TC], ps[:]
                        )
                    nc.sync.dma_start(out[tb * 128:(tb + 1) * 128, :], ot[:])

            def gather_recip(j):
                # off the critical path: runs during unit j+1's matmuls
                dcomp = dcp.tile([8, TC], F32, tag="dc", name="dc")
                for b3 in range(3):
                    lo, n = 3 * b3, min(3, 8 - 3 * b3)
                    nc.sync.dma_start(
                        dcomp[lo:lo + n, :],
                        dens[32 * b3:32 * b3 + 1, 3 * j:3 * j + n, :],
                    )
                rec = dcp.tile([8, TC], F32R, tag="rec", name="rec")
                with nc.allow_low_precision(
                    reason="fp32r reciprocal feeds bcast matmul"
                ):
                    nc.vector.reciprocal(rec[:], dcomp[:])
                recs[j] = rec

            for t in range(N_TC):
                if t >= 1:
                    gather_recip(t - 1)
                # -------- phase 2 unit: project token chunk t ------------
                xt = xts[t]
                if t + 1 < N_TC:
                    nc.sync.dma_start(
                        xts[t + 1][:],
                        xT[:, (t + 1) * TC:(t + 2) * TC].rearrange(
                            "(kc p) n -> p kc n", p=128
                        ),
                    )
                for oc in range(N_GCB):      # QT and KT column blocks
                    for w_s, dst, eng in (
                        (wqs, qt, nc.scalar), (wks, kt, nc.vector)
                    ):
                        ps = pjp.tile([128, TC], F32, tag="pj", name="pj")
                        for kc in range(N_CC):
                            nc.tensor.matmul(
                                ps[:],
                                w_s[:, kc, oc * 128:(oc + 1) * 128],
                                xt[:, kc, :],
                                start=(kc == 0),
                                stop=(kc == N_CC - 1),
                            )
                        dslc = dst[:, oc, t * TC:(t + 1) * TC]
                        if eng is nc.scalar:
                            nc.scalar.copy(dslc, ps[:])
                        else:
                            nc.vector.tensor_copy(dslc, ps[:])
                for tb in range(TC // 128):  # V token blocks
                    ps = pjp.tile([128, GC], F32, tag="pj", name="pj")
                    for kc in range(N_CC):
                        nc.tensor.matmul(
                            ps[:],
                            xt[:, kc, tb * 128:(tb + 1) * 128],
                            wvs[:, kc, :],
                            start=(kc == 0),
                            stop=(kc == N_CC - 1),
                        )
                    nc.vector.tensor_copy(
                        vaug[:, t * 4 + tb, :, :Dh],
                        ps.rearrange("p (h d) -> p h d", h=HPG),
                    )

                # -------- phase 3 unit: attention for query chunk j=t ----
                j = t
                qslc = slice(j * TC, (j + 1) * TC)
                for p in range(HPG // 2):    # head pairs: rows 0:64 / 64:128
                    avs = [
                        avp.tile([Dh + 1, TC], F32, tag="av", name=f"av{i}")
                        for i in range(2)
                    ]
                    nkb = 4 * j + 4
                    for kb in range(nkb):
                        m = kb - 4 * j       # >=0 on diagonal blocks
                        c0 = m * 128 if m > 0 else 0
                        w = TC - c0          # live query columns per half
                        # both heads' score tiles packed [c0:512 | 512:512+w]
                        # in one 2-bank PSUM tile -> single exp op per kb
                        sc = scp.tile([128, 2 * TC], F32, tag="sc", name="sc")
                        for half in range(2):
                            p0 = half * Dh
                            dst = (
                                sc[:, c0:TC] if half == 0
                                else sc[:, TC:TC + w]
                            )
                            nc.tensor.matmul(
                                dst,
                                kt[p0:p0 + Dh, p, kb * KB:(kb + 1) * KB],
                                qt[p0:p0 + Dh, p, j * TC + c0:(j + 1) * TC],
                                start=True,
                                stop=True,
                            )
                        pr = prp.tile([128, 2 * TC], BF16, tag="pr", name="pr")
                        nc.scalar.activation(
                            pr[:, c0:TC + w], sc[:, c0:TC + w],
                            mybir.ActivationFunctionType.Exp,
                        )
                        if m >= 0:
                            # causal triangle on the leading 128 live cols
                            for half in range(2):
                                base = c0 if half == 0 else TC
                                nc.vector.tensor_mul(
                                    pr[:, base:base + KB],
                                    pr[:, base:base + KB],
                                    msk[:],
                                )
                        for half in range(2):
                            src = (
                                pr[:, c0:TC] if half == 0
                                else pr[:, TC:TC + w]
                            )
                            nc.tensor.matmul(
                                avs[half][:, c0:],
                                vaug[:, kb, 2 * p + half, :],
                                src,
                                start=(kb == 0),
                                stop=(kb == nkb - 1),
                            )
                    for half in range(2):
                        p0 = half * Dh
                        idx8 = 2 * p + half
                        nc.vector.tensor_copy(
                            aot[p0:p0 + Dh, p, qslc], avs[half][:Dh, :]
                        )
                        db, dc = 32 * (idx8 // 3), 3 * j + idx8 % 3
                        nc.vector.tensor_copy(
                            dens[db:db + 1, dc, :], avs[half][Dh:Dh + 1, :]
                        )

                if t >= 1:
                    norm_outproj(t - 1)

            gather_recip(N_TC - 1)
            norm_outproj(N_TC - 1)

    nc.compile()
    return nc


_CACHE = {}


def _make_mask():
    m = np.zeros((KB, KB), np.float32)
    for dk in range(KB):
        m[dk, dk:] = 1.0
    return m.astype(_BF)


def _make_sel():
    s = np.zeros((8, 8 * Dh), np.float32)
    for i in range(8):
        s[i, i * Dh:(i + 1) * Dh] = 1.0
    return s


def make_in_maps(x, W_qkv, W_out):
    mask = _make_mask()
    sel = _make_sel()
    in_maps = []
    for core in range(N_CORES):
        b, g = divmod(core, G)
        cs = slice(g * GC, (g + 1) * GC)
        in_maps.append({
            "xT": np.ascontiguousarray(x[b].T).astype(_BF),
            "wq": np.ascontiguousarray(W_qkv[:, cs] * 0.125).astype(_BF),
            "wk": np.ascontiguousarray(
                W_qkv[:, C + g * GC:C + (g + 1) * GC]).astype(_BF),
            "wv": np.ascontiguousarray(
                W_qkv[:, 2 * C + g * GC:2 * C + (g + 1) * GC]).astype(_BF),
            "wo": np.ascontiguousarray(W_out[cs, :]).astype(_BF),
            "mask": mask,
            "sel": sel,
        })
    return in_maps


def kernel(x, W_qkv, W_out):
    x = np.ascontiguousarray(np.asarray(x, dtype=np.float32))
    W_qkv = np.asarray(W_qkv, dtype=np.float32)
    W_out = np.asarray(W_out, dtype=np.float32)

    if "nc" not in _CACHE:
        _CACHE["nc"] = build_program()
    nc = _CACHE["nc"]

    in_maps = make_in_maps(x, W_qkv, W_out)
    res = bass_utils.run_bass_kernel_spmd(nc, in_maps, core_ids=list(range(N_CORES)))

    out = np.empty((B, T, C), np.float32)
    for b in range(B):
        out[b] = res.results[G * b]["out"].astype(np.float32)
        for g in range(1, G):
            out[b] += res.results[G * b + g]["out"].astype(np.float32)
    return out


# revision 10
# speedup vs baseline: 1.3114x; 1.2031x over previous
"""Multi-head causal attention on 8 Trainium2 NeuronCores.

Sharding: data-parallel over batch (4) x tensor-parallel over heads (2 groups
of 8 heads). Each core computes a partial output [T, C] for one batch element
using its 8 heads; the host sums the two partials per batch element (the
"all-reduce after out_proj" done during unshard).

v3 structure: the kernel is a single loop over 4 token chunks.  Unit t
projects chunk t (Q/K/V), runs attention for query chunk j=t (whose keys are
exactly chunks 0..t), and then runs softmax-normalization + out_proj for
chunk j=t-1 (software pipelining: the denominator-gather DMA + reciprocal of
chunk j run during unit j+1's matmuls, so the PE never waits on them).
Causal raggedness: for query chunk j, diagonal key block kb=4j+m streams
only the N = 512-128m live query columns, exp covers only the live range,
and the causal mask is one [128,128] triangle multiply per diagonal block.
Output is DMA'd bf16 (host sums the two group partials in f32).

Per-core layouts (no on-device transposes needed):
  inputs: xT [C, T] (x[b] transposed on host), Wq*0.125/Wk/Wv [C, 512],
          Wo [512, C], triangle mask [128, 128] bf16, sel [8, 512] f32r.
  QT = (Wq/8)^T @ x^T  [512, T]   (lhsT = Wq chunk, rhs = xT chunk)
  KT = Wk^T @ x^T      [512, T]
  V  = x @ Wv          [T, 512]   ones-augmented as vaug [T, 8 heads, 65]
  per head-pair p, query-chunk j, key-block kb (ragged causal):
     sT  = K_h[kb]^T @ Q_h[:, live]      [128, <=512] PSUM per half
     p   = exp(sT)  (skip-max softmax: |s| < ~9), one exp per kb tile
     p  *= triangle mask on the leading 128 cols of diagonal blocks
     av += V_aug[kb, h]^T @ p            [65, 512] PSUM; row 64 = denom
  per j: gather the 8 denom rows, reciprocal_approx_fast, sel-matmul
  broadcast, normalize aot in place, out_proj of chunk j -> bf16 DMA out.
"""

import numpy as np
import ml_dtypes

_BF = ml_dtypes.bfloat16

import concourse.bass as bass
import concourse.bacc as bacc
import concourse.mybir as mybir
import concourse.tile as tile
from concourse import bass_utils

F32 = mybir.dt.float32
F32R = mybir.dt.float32r
BF16 = mybir.dt.bfloat16

B, T, C = 4, 2048, 1024
H, Dh = 16, 64
G = 2                 # head groups (tensor parallel)
HPG = H // G          # heads per group
GC = HPG * Dh         # group channels = 512
N_CORES = 8
TC = 512              # token chunk (projection and query chunks)
KB = 128              # key block
N_TC = T // TC        # 4
N_KB = T // KB        # 16
N_CC = C // 128       # contraction chunks over C = 8
N_GCB = GC // 128     # chan blocks in a group = 4


def build_program():
    nc = bacc.Bacc("TRN2", target_bir_lowering=False, debug=False)

    xT = nc.dram_tensor("xT", [C, T], BF16, kind="ExternalInput").ap()
    wq = nc.dram_tensor("wq", [C, GC], BF16, kind="ExternalInput").ap()
    wk = nc.dram_tensor("wk", [C, GC], BF16, kind="ExternalInput").ap()
    wv = nc.dram_tensor("wv", [C, GC], BF16, kind="ExternalInput").ap()
    wo = nc.dram_tensor("wo", [GC, C], BF16, kind="ExternalInput").ap()
    mask_in = nc.dram_tensor("mask", [KB, KB], BF16, kind="ExternalInput").ap()
    sel_in = nc.dram_tensor("sel", [8, 8 * Dh], F32R, kind="ExternalInput").ap()
    out = nc.dram_tensor("out", [T, C], BF16, kind="ExternalOutput").ap()

    with tile.TileContext(nc) as tc:
        with (
            tc.tile_pool(name="persist", bufs=1) as pp,
            tc.tile_pool(name="x_pool", bufs=2) as xp,
            tc.tile_pool(name="probs", bufs=4) as prp,
            tc.tile_pool(name="outs", bufs=2) as otp,
            tc.tile_pool(name="dc_pool", bufs=2) as dcp,
            tc.tile_pool(name="pj_psum", bufs=2, space="PSUM") as pjp,
            tc.tile_pool(name="sc_psum", bufs=2, space="PSUM") as scp,
            tc.tile_pool(name="av_psum", bufs=2, space="PSUM") as avp,
        ):
            qt = pp.tile([128, N_GCB, T], BF16)        # QT (chan%128, chan//128, tok)
            kt = pp.tile([128, N_GCB, T], BF16)
            vaug = pp.tile([128, N_KB, HPG, Dh + 1], BF16)
            aot = pp.tile([128, N_GCB, T], BF16)       # attn_outT
            msk = pp.tile([128, KB], BF16)
            sel = pp.tile([8, 8 * Dh], F32R)
            # softmax denominator staging: slot (j, idx8) at partition
            # 32*(idx8//3), column 3*j + idx8%3 (engine APs start at 0/32/64)
            dens = pp.tile([128, 12, TC], F32)

            wqs = pp.tile([128, N_CC, GC], BF16)
            wks = pp.tile([128, N_CC, GC], BF16)
            wvs = pp.tile([128, N_CC, GC], BF16)
            wos = pp.tile([128, N_GCB, C], BF16)

            xts = []
            for t in range(N_TC):
                xts.append(
                    xp.tile([128, N_CC, TC], BF16, tag="xt", name=f"xt{t}")
                )

            # startup DMAs spread across engine queues so the first
            # transfers run concurrently
            nc.sync.dma_start(wqs[:], wq.rearrange("(kc p) n -> p kc n", p=128))
            nc.sync.dma_start(
                xts[0][:], xT[:, 0:TC].rearrange("(kc p) n -> p kc n", p=128)
            )
            nc.sync.dma_start(wks[:], wk.rearrange("(kc p) n -> p kc n", p=128))
            nc.sync.dma_start(wvs[:], wv.rearrange("(kc p) n -> p kc n", p=128))
            nc.sync.dma_start(msk[:], mask_in)
            nc.sync.dma_start(sel[:], sel_in)
            nc.sync.dma_start(wos[:], wo.rearrange("(cb p) n -> p cb n", p=128))
            nc.vector.memset(vaug[:, :, :, Dh:], 1.0)

            recs = {}

            def norm_outproj(j):
                qslc = slice(j * TC, (j + 1) * TC)
                rec = recs.pop(j)
                for p in range(HPG // 2):
                    for half in range(2):
                        p0 = half * Dh
                        idx8 = 2 * p + half
                        bc = pjp.tile([Dh, TC], F32, tag="pj", name="bc")
                        nc.tensor.matmul(
                            bc[:],
                            sel[:, idx8 * Dh:(idx8 + 1) * Dh],
                            rec[:],
                            start=True, stop=True,
                        )
                        nc.vector.tensor_mul(
                            aot[p0:p0 + Dh, p, qslc],
                            aot[p0:p0 + Dh, p, qslc],
                            bc[:],
                        )
                for tb in range(4 * j, 4 * j + 4):
                    ot = otp.tile([128, C], BF16, tag="ot", name="ot")
                    for oc in range(C // TC):
                        ps = pjp.tile([128, TC], F32, tag="pj", name="op")
                        for cc in range(N_GCB):
                            nc.tensor.matmul(
                                ps[:],
                                aot[:, cc, tb * 128:(tb + 1) * 128],
                                wos[:, cc, oc * TC:(oc + 1) * TC],
                                start=(cc == 0),
                                stop=(cc == N_GCB - 1),
                            )
                        nc.vector.tensor_copy(
                            ot[:, oc * TC:(oc + 1) * TC], ps[:]
                        )
                    nc.sync.dma_start(out[tb * 128:(tb + 1) * 128, :], ot[:])

            def gather_recip(j):
                # off the critical path: runs during unit j+1's matmuls
                dcomp = dcp.tile([8, TC], F32, tag="dc", name="dc")
                for b3 in range(3):
                    lo, n = 3 * b3, min(3, 8 - 3 * b3)
                    nc.sync.dma_start(
                        dcomp[lo:lo + n, :],
                        dens[32 * b3:32 * b3 + 1, 3 * j:3 * j + n, :],
                    )
                rec = dcp.tile([8, TC], F32R, tag="rec", name="rec")
                with nc.allow_low_precision(
                    reason="fp32r reciprocal feeds bcast matmul"
                ):
                    nc.vector.reciprocal(rec[:], dcomp[:])
                recs[j] = rec

            for t in range(N_TC):
                if t >= 1:
                    gather_recip(t - 1)
                # -------- phase 2 unit: project token chunk t ------------
                xt = xts[t]
                if t + 1 < N_TC:
                    nc.sync.dma_start(
                        xts[t + 1][:],
                        xT[:, (t + 1) * TC:(t + 2) * TC].rearrange(
                            "(kc p) n -> p kc n", p=128
                        ),
                    )
                for oc in range(N_GCB):      # QT and KT column blocks
                    for w_s, dst, eng in (
                        (wqs, qt, nc.scalar), (wks, kt, nc.vector)
                    ):
                        ps = pjp.tile([128, TC], F32, tag="pj", name="pj")
                        for kc in range(N_CC):
                            nc.tensor.matmul(
                                ps[:],
                                w_s[:, kc, oc * 128:(oc + 1) * 128],
                                xt[:, kc, :],
                                start=(kc == 0),
                                stop=(kc == N_CC - 1),
                            )
                        dslc = dst[:, oc, t * TC:(t + 1) * TC]
                        if eng is nc.scalar:
                            nc.scalar.copy(dslc, ps[:])
                        else:
                            nc.vector.tensor_copy(dslc, ps[:])
                for tb in range(TC // 128):  # V token blocks
                    ps = pjp.tile([128, GC], F32, tag="pj", name="pj")
                    for kc in range(N_CC):
                        nc.tensor.matmul(
                            ps[:],
                            xt[:, kc, tb * 128:(tb + 1) * 128],
                            wvs[:, kc, :],
                            start=(kc == 0),
                            stop=(kc == N_CC - 1),
                        )
                    nc.vector.tensor_copy(
                        vaug[:, t * 4 + tb, :, :Dh],
                        ps.rearrange("p (h d) -> p h d", h=HPG),
                    )

                # -------- phase 3 unit: attention for query chunk j=t ----
                j = t
                qslc = slice(j * TC, (j + 1) * TC)
                for p in range(HPG // 2):    # head pairs: rows 0:64 / 64:128
                    avs = [
                        avp.tile([Dh + 1, TC], F32, tag="av", name=f"av{i}")
                        for i in range(2)
                    ]
                    nkb = 4 * j + 4
                    for kb in range(nkb):
                        m = kb - 4 * j       # >=0 on diagonal blocks
                        c0 = m * 128 if m > 0 else 0
                        w = TC - c0          # live query columns per half
                        # both heads' score tiles packed [c0:512 | 512:512+w]
                        # in one 2-bank PSUM tile -> single exp op per kb
                        sc = scp.tile([128, 2 * TC], F32, tag="sc", name="sc")
                        for half in range(2):
                            p0 = half * Dh
                            dst = (
                                sc[:, c0:TC] if half == 0
                                else sc[:, TC:TC + w]
                            )
                            nc.tensor.matmul(
                                dst,
                                kt[p0:p0 + Dh, p, kb * KB:(kb + 1) * KB],
                                qt[p0:p0 + Dh, p, j * TC + c0:(j + 1) * TC],
                                start=True,
                                stop=True,
                            )
                        pr = prp.tile([128, 2 * TC], BF16, tag="pr", name="pr")
                        nc.scalar.activation(
                            pr[:, c0:TC + w], sc[:, c0:TC + w],
                            mybir.ActivationFunctionType.Exp,
                        )
                        if m >= 0:
                            # causal triangle on the leading 128 live cols
                            for half in range(2):
                                base = c0 if half == 0 else TC
                                nc.vector.tensor_mul(
                                    pr[:, base:base + KB],
                                    pr[:, base:base + KB],
                                    msk[:],
                                )
                        for half in range(2):
                            src = (
                                pr[:, c0:TC] if half == 0
                                else pr[:, TC:TC + w]
                            )
                            nc.tensor.matmul(
                                avs[half][:, c0:],
                                vaug[:, kb, 2 * p + half, :],
                                src,
                                start=(kb == 0),
                                stop=(kb == nkb - 1),
                            )
                    for half in range(2):
                        p0 = half * Dh
                        idx8 = 2 * p + half
                        nc.vector.tensor_copy(
                            aot[p0:p0 + Dh, p, qslc], avs[half][:Dh, :]
                        )
                        db, dc = 32 * (idx8 // 3), 3 * j + idx8 % 3
                        nc.vector.tensor_copy(
                            dens[db:db + 1, dc, :], avs[half][Dh:Dh + 1, :]
                        )

                if t >= 1:
                    norm_outproj(t - 1)

            gather_recip(N_TC - 1)
            norm_outproj(N_TC - 1)

    nc.compile()
    return nc


_CACHE = {}


def _make_mask():
    m = np.zeros((KB, KB), np.float32)
    for dk in range(KB):
        m[dk, dk:] = 1.0
    return m.astype(_BF)


def _make_sel():
    s = np.zeros((8, 8 * Dh), np.float32)
    for i in range(8):
        s[i, i * Dh:(i + 1) * Dh] = 1.0
    return s


def make_in_maps(x, W_qkv, W_out):
    mask = _make_mask()
    sel = _make_sel()
    in_maps = []
    for core in range(N_CORES):
        b, g = divmod(core, G)
        cs = slice(g * GC, (g + 1) * GC)
        in_maps.append({
            "xT": np.ascontiguousarray(x[b].T).astype(_BF),
            "wq": np.ascontiguousarray(W_qkv[:, cs] * 0.125).astype(_BF),
            "wk": np.ascontiguousarray(
                W_qkv[:, C + g * GC:C + (g + 1) * GC]).astype(_BF),
            "wv": np.ascontiguousarray(
                W_qkv[:, 2 * C + g * GC:2 * C + (g + 1) * GC]).astype(_BF),
            "wo": np.ascontiguousarray(W_out[cs, :]).astype(_BF),
            "mask": mask,
            "sel": sel,
        })
    return in_maps


def kernel(x, W_qkv, W_out):
    x = np.ascontiguousarray(np.asarray(x, dtype=np.float32))
    W_qkv = np.asarray(W_qkv, dtype=np.float32)
    W_out = np.asarray(W_out, dtype=np.float32)

    if "nc" not in _CACHE:
        _CACHE["nc"] = build_program()
    nc = _CACHE["nc"]

    in_maps = make_in_maps(x, W_qkv, W_out)
    res = bass_utils.run_bass_kernel_spmd(nc, in_maps, core_ids=list(range(N_CORES)))

    out = np.empty((B, T, C), np.float32)
    for b in range(B):
        out[b] = res.results[G * b]["out"].astype(np.float32)
        for g in range(1, G):
            out[b] += res.results[G * b + g]["out"].astype(np.float32)
    return out
